# revision 13
# baseline (speedup 1.0000x reference)
"""Trainium2 Bass kernel for nn_DFPT_Node (soft binary decision tree).

Full inputs in, full output out; internally data-parallel over 8 NeuronCores
(batch sharded 65536 -> 8 x 8192). Tree params (c, s, dims, leaf_logits) are
baked into compiled constants on the host.

  gate:  g = sigmoid(-4 (x[:,dims] - c)/|s|) = sigmoid(a*x + b) via a scaled
         one-hot matmul with K=128 = [x_hi(64) | x_lo(62 dims) | 1 | 1]; the
         last two rows carry b = b_hi + b_lo (f16 split, ~22 bits), so the
         sigmoid needs no per-chunk bias and one ACT instruction can span
         chunk boundaries (1536-wide supertiles, fewer ACT init charges).
         The two dims whose sharpest gate is softest lose their x_lo row
         (slope <= ~20, error ~1e-3 in z; harmless).
  tree:  levels 0-6 batch-major (batch on partitions), levels 7-9 node-major
         (nodes on partitions, batch on free dim) in block (bit-reversed)
         leaf order; level 9 folded into the output matmul with an 8-chunk
         basis F = [l8, r8, l9a, l9b, u0, u1, q2, q4] (q2 = r9a*g, q4 =
         r9b*g via explicit r9a/r9b subtractions - 2 fewer PSUM chunks than
         the 10-chunk basis at the same DVE op count).
  sched: event-driven software pipeline at chunk granularity: each sigmoid
         supertile completion triggers exactly the newly-unblocked shallow /
         deep / fold work, so the drain after the last sigmoid is short.

Output leaves the device as outT [10->16, B_core] packed 4 slabs per 128
partitions; host transposes back.
"""

import numpy as np

B_FULL = 65536
IN_DIM = 64
N_CLASS = 10
MAX_DEPTH = 10
N_CORES = 8
B_CORE = B_FULL // N_CORES      # 8192
SLAB = 1024                     # batch columns per slab
N_SLABS = B_CORE // SLAB        # 8
N_CHUNKS = 8                    # node-major chunks of 128 nodes
N_FCHUNKS = 8                   # fold basis chunks
SUPER = 1536                    # sigmoid supertile width (3 psum banks)
PAIR_FLAT = 2 * N_CHUNKS * SLAB          # 16384 flat gt elems per slab pair
STEPS_PER_PAIR = (PAIR_FLAT + SUPER - 1) // SUPER  # 11
N_PAIRS = N_SLABS // 2

F16 = np.float16
F32 = np.float32

_CACHE = {}


def _build_tree_layout():
    """pos[d][i] = reference position within level d of block-order index i."""
    pos = [np.array([0], dtype=np.int64)]
    for _ in range(MAX_DEPTH):
        p = pos[-1]
        pos.append(np.concatenate([2 * p, 2 * p + 1]))
    return pos


def _build_constants(c, s, dims, leaf_logits):
    """W chunks [8,128,128] f16 (bias folded in rows 126/127), M [8,128,10]."""
    pos = _build_tree_layout()
    chunk_nodes = -np.ones((N_CHUNKS, 128), dtype=np.int64)
    for d in range(7):
        base = (1 << d) - 1
        chunk_nodes[0, base: base + (1 << d)] = base + pos[d]
    chunk_nodes[1, :] = 127 + pos[7]
    lvl8 = 255 + pos[8]
    chunk_nodes[2, :] = lvl8[:128]
    chunk_nodes[3, :] = lvl8[128:]
    lvl9 = 511 + pos[9]
    for t in range(4):
        chunk_nodes[4 + t, :] = lvl9[128 * t: 128 * (t + 1)]

    a64 = -4.0 / np.abs(s.astype(np.float64))
    a16 = a64.astype(F16)
    b64 = -a16.astype(np.float64) * c.astype(np.float64)
    b_hi = b64.astype(F16)
    b_lo = (b64 - b_hi.astype(np.float64)).astype(F16)

    # the two dims whose sharpest gate is softest lose their x_lo row
    min_s = np.full(IN_DIM, np.inf)
    for g in range(len(dims)):
        d = int(dims[g])
        min_s[d] = min(min_s[d], abs(float(s[g])))
    drop = np.argsort(-min_s)[:2]
    lo_row = {}
    r = IN_DIM
    for d in range(IN_DIM):
        if d not in drop:
            lo_row[d] = r
            r += 1
    assert r == 126

    W = np.zeros((N_CHUNKS, 128, 128), dtype=F16)
    ch_idx, col_idx = np.nonzero(chunk_nodes >= 0)
    g_idx = chunk_nodes[ch_idx, col_idx]
    for ch, col, g in zip(ch_idx, col_idx, g_idx):
        d = int(dims[g])
        W[ch, d, col] = a16[g]
        if d in lo_row:
            W[ch, lo_row[d], col] = a16[g]
        W[ch, 126, col] = b_hi[g]
        W[ch, 127, col] = b_lo[g]

    L_my = leaf_logits[pos[MAX_DEPTH]].astype(np.float64)  # [1024, 10] block
    A = L_my[:512] - L_my[512:]
    Bm = L_my[512:]
    At = [A[128 * t: 128 * (t + 1)] for t in range(4)]
    Bt = [Bm[128 * t: 128 * (t + 1)] for t in range(4)]
    # F basis: [l8, r8, l9a, l9b, u0=l9a*g9a, u1=l9b*g9b, q2=r9a*g9c,
    #           q4=r9b*g9d] with r9a = l8-l9a, r9b = r8-l9b:
    # out = l8 B2 + r8 B3 + l9a (B0-B2) + l9b (B1-B3) + u0 A0 + u1 A1
    #       + q2 A2 + q4 A3
    Mlist = [Bt[2], Bt[3], Bt[0] - Bt[2], Bt[1] - Bt[3],
             At[0], At[1], At[2], At[3]]
    M = np.zeros((N_FCHUNKS, 128, N_CLASS), dtype=F16)
    for i, m in enumerate(Mlist):
        M[i] = m.astype(F16)
    return W, M, lo_row


def _step_table():
    """Per sigmoid step: (gt dest, offset, width, z pieces, completions).

    Phase 1 evaluates chunk 0 (shallow gates) of every slab into gt0 so all
    cascades and transposes run early; phase 2 evaluates chunks 1-7
    slab-major into per-pair gtd tiles. A piece is 512 batch columns
    [h*512, h*512+512) of one chunk of one slab.
    """
    steps = []
    p = 0
    off = 0
    for w in (512, 1536, 1536, 1536, 1536, 1536):
        n = w // 512
        pieces = [(q // 2, 0, q % 2) for q in range(p, p + n)]
        done = [(q // 2, 0) for q in range(p, p + n) if q % 2 == 1]
        steps.append(("gt0", None, off, w, pieces, done))
        p += n
        off += w
    for pair in range(N_PAIRS):
        p = 0
        off = 0
        for k, w in enumerate([1536] * 9 + [512]):
            n = w // 512
            pieces = []
            done = []
            for q in range(p, p + n):
                s_in, r = divmod(q, 14)
                s = 2 * pair + s_in
                ch, h = 1 + r // 2, r % 2
                pieces.append((s, ch, h))
                if h == 1:
                    done.append((s, ch))
            steps.append(("gtd", pair, off, w, pieces, done))
            p += n
            off += w
    return steps


def _build_program():
    import concourse.bass as bass
    import concourse.tile as tile
    from concourse import bacc, mybir

    f16 = mybir.dt.float16
    f32 = mybir.dt.float32
    SIG = mybir.ActivationFunctionType.Sigmoid

    nc = bacc.Bacc("TRN2", target_bir_lowering=False)
    xt2_d = nc.dram_tensor("xt2", [128, B_CORE], f16, kind="ExternalInput")
    wt_d = nc.dram_tensor("wt", [128, N_CHUNKS, 128], f16, kind="ExternalInput")
    mt_d = nc.dram_tensor("mt", [128, N_FCHUNKS, 16], f16, kind="ExternalInput")
    out_d = nc.dram_tensor("outt", [128, B_CORE // 4], f16, kind="ExternalOutput")

    steps = _step_table()

    with tile.TileContext(nc) as tc:
        with (
            tc.tile_pool(name="singles", bufs=1) as singles,
            tc.tile_pool(name="gtpool", bufs=2) as gtpool,
            tc.tile_pool(name="work", bufs=2) as work,
            tc.tile_pool(name="fwork", bufs=2) as fwork,
            tc.tile_pool(name="zpsum", bufs=2, space="PSUM") as zpsum,
            tc.tile_pool(name="opsum", bufs=1, space="PSUM") as opsum,
        ):
            # resident constants; ordered so step 0's z can start ASAP
            w_sb = singles.tile([128, N_CHUNKS, 128], f16)
            nc.sync.dma_start(out=w_sb[:, 0:1, :], in_=wt_d[:, 0:1, :])
            xt2 = singles.tile([128, B_CORE], f16)
            nc.sync.dma_start(out=xt2[:, 0:512], in_=xt2_d[:, 0:512])
            nc.sync.dma_start(out=xt2[:, 512:SLAB], in_=xt2_d[:, 512:SLAB])
            for sl in range(1, N_SLABS):
                t = bass.ts(sl, SLAB)
                nc.sync.dma_start(out=xt2[:, t], in_=xt2_d[:, t])
            nc.sync.dma_start(out=w_sb[:, 1:, :], in_=wt_d[:, 1:, :])
            ones0 = singles.tile([128, 8, 1], f16)
            nc.vector.memset(ones0, 1.0)
            sigwarm = singles.tile([1, 1], f16)
            nc.vector.memset(sigwarm, 0.0)
            nc.scalar.activation(sigwarm, sigwarm, SIG, bias=0.0, scale=1.0)
            m_sb = singles.tile([128, N_FCHUNKS, 16], f16)
            nc.sync.dma_start(out=m_sb, in_=mt_d[:, :, :])
            gt0 = singles.tile([128, N_SLABS * SLAB], f16)

            state = {}

            def emit_shallow(s):
                # chunk-0 gates batch-major via DMA xbar transpose
                gsh = work.tile([128, 8, 128], f16, tag="gsh", name=f"gsh{s}")
                for g in range(8):
                    nc.sync.dma_start(
                        out=gsh[:, g, :],
                        in_=gt0[:, s * SLAB + g * 128: s * SLAB + (g + 1) * 128],
                        transpose=True,
                    )
                # shallow cascade (batch-major, block layout, groups stacked)
                prev = ones0[:, 0:8, :]
                for d in range(7):
                    n = 1 << d
                    cur = work.tile([128, 8, 2 * n], f16, tag=f"pb{d + 1}",
                                    name=f"pb{d + 1}_{s}")
                    gl = gsh[:, :, n - 1: 2 * n - 1]
                    nc.vector.tensor_mul(cur[:, :, 0:n], prev, gl)
                    nc.vector.tensor_sub(cur[:, :, n: 2 * n], prev,
                                         cur[:, :, 0:n])
                    prev = cur
                # p7 batch-major -> node-major via DMA xbar transpose
                p7t = work.tile([128, SLAB], f16, tag="p7t", name=f"p7t{s}")
                for g in range(8):
                    nc.sync.dma_start(
                        out=p7t[:, g * 128: (g + 1) * 128],
                        in_=prev[:, g, :],
                        transpose=True,
                    )
                state[("p7t", s)] = p7t

            def gtc(s, ch):
                gt = state[("gtd", s // 2)]
                base = ((s % 2) * 7 + ch - 1) * SLAB
                return gt[:, base: base + SLAB]

            def ft(name, s):
                t = fwork.tile([128, SLAB], f16, tag=name, name=f"{name}_{s}")
                state[(name, s)] = t
                return t

            FNAMES = ("l8", "r8", "l9a", "l9b", "u0", "u1", "q2", "q4")

            def emit_fold(s, f):
                # fold chunk f of slab s into op[32j:32j+16, h*512:+512]
                grp, j = divmod(s, 4)
                if f == 0 and j == 0:
                    state["op"] = opsum.tile([128, SLAB], f32, tag="op",
                                             name=f"op{grp}")
                op = state["op"]
                src = state[(FNAMES[f], s)]
                for h in range(2):
                    nc.tensor.matmul(
                        op[32 * j: 32 * j + 16, h * 512: h * 512 + 512],
                        lhsT=m_sb[:, f, :],
                        rhs=src[:, h * 512: h * 512 + 512],
                        start=(f == 0),
                        stop=(f == N_FCHUNKS - 1),
                        tile_position=(0, 32 * j),
                    )

            def emit_early_out(grp):
                # slabs 4g..4g+2 (psum rows 0:96) are folded; copy + store
                # them while slab 4g+3 still accumulates rows 96:128
                op = state["op"]
                osb = work.tile([128, SLAB], f16, tag="osb", name=f"osb{grp}")
                state[("osb", grp)] = osb
                nc.vector.tensor_copy(osb[0:96, :], op[0:96, :])
                nc.sync.dma_start(
                    out=out_d[0:96, grp * SLAB: (grp + 1) * SLAB],
                    in_=osb[0:96, :],
                )

            def emit_group_out(grp):
                op = state["op"]
                osb = state[("osb", grp)]
                nc.vector.tensor_copy(osb[96:128, 0:512], op[96:128, 0:512])
                if grp == N_SLABS // 4 - 1:
                    nc.scalar.copy(osb[96:128, 512:1024], op[96:128, 512:1024])
                else:
                    nc.vector.tensor_copy(osb[96:128, 512:1024],
                                          op[96:128, 512:1024])
                nc.sync.dma_start(
                    out=out_d[96:128, grp * SLAB: (grp + 1) * SLAB],
                    in_=osb[96:128, :],
                )

            def handle(s, ch):
                if ch == 0:
                    emit_shallow(s)
                elif ch == 1:
                    p7t = state[("p7t", s)]
                    l8 = ft("l8", s)
                    nc.vector.tensor_mul(l8, p7t, gtc(s, 1))
                    r8 = ft("r8", s)
                    nc.vector.tensor_sub(r8, p7t, l8)
                elif ch == 2:
                    l8 = state[("l8", s)]
                    l9a = ft("l9a", s)
                    nc.vector.tensor_mul(l9a, l8, gtc(s, 2))
                    r9a = ft("r9a", s)
                    nc.vector.tensor_sub(r9a, l8, l9a)
                    emit_fold(s, 0)
                    emit_fold(s, 1)
                elif ch == 3:
                    r8 = state[("r8", s)]
                    l9b = ft("l9b", s)
                    nc.vector.tensor_mul(l9b, r8, gtc(s, 3))
                    r9b = ft("r9b", s)
                    nc.vector.tensor_sub(r9b, r8, l9b)
                    emit_fold(s, 2)
                elif ch == 4:
                    u0 = ft("u0", s)
                    nc.gpsimd.tensor_mul(u0, state[("l9a", s)], gtc(s, 4))
                    emit_fold(s, 3)
                elif ch == 5:
                    u1 = ft("u1", s)
                    nc.gpsimd.tensor_mul(u1, state[("l9b", s)], gtc(s, 5))
                    emit_fold(s, 4)
                elif ch == 6:
                    q2 = ft("q2", s)
                    nc.vector.tensor_mul(q2, state[("r9a", s)], gtc(s, 6))
                    emit_fold(s, 5)
                elif ch == 7:
                    q4 = ft("q4", s)
                    nc.vector.tensor_mul(q4, state[("r9b", s)], gtc(s, 7))
                    emit_fold(s, 6)
                    state["fold7"] = s

            def flush_fold7():
                s = state.pop("fold7", None)
                if s is not None:
                    emit_fold(s, 7)
                    if s % 4 == 2:
                        emit_early_out(s // 4)
                    elif s % 4 == 3:
                        emit_group_out(s // 4)

            for ti, (dest, pair, off, width, pieces, done) in enumerate(steps):
                with tc.high_priority():
                    if dest == "gt0":
                        gt = gt0
                    else:
                        if off == 0:
                            state[("gtd", pair)] = gtpool.tile(
                                [128, 2 * 7 * SLAB], f16, tag="gtd",
                                name=f"gtd{pair}")
                        gt = state[("gtd", pair)]
                    zs = zpsum.tile([128, SUPER], f32, tag="zs",
                                    name=f"zs{ti}")
                    for i, (s, ch, h) in enumerate(pieces):
                        col0 = s * SLAB + h * 512
                        nc.tensor.matmul(
                            zs[:, i * 512: (i + 1) * 512],
                            lhsT=w_sb[:, ch, :],
                            rhs=xt2[:, col0: col0 + 512],
                            start=True,
                            stop=True,
                        )
                    nc.scalar.activation(
                        gt[:, off: off + width],
                        zs[:, 0:width], SIG, bias=0.0, scale=1.0,
                    )
                flush_fold7()
                for s, ch in done:
                    handle(s, ch)
            flush_fold7()

    nc.finalize()
    return nc


def _get_program():
    if "nc" not in _CACHE:
        _CACHE["nc"] = _build_program()
    return _CACHE["nc"]


def kernel(x, c, s, leaf_logits, dims, max_depth):
    from concourse.bass_utils import run_bass_kernel_spmd

    assert int(max_depth) == MAX_DEPTH
    x = np.asarray(x, dtype=F32)
    c = np.asarray(c, dtype=F32)
    s = np.asarray(s, dtype=F32)
    leaf_logits = np.asarray(leaf_logits, dtype=F32)
    dims = np.asarray(dims)

    W, M, lo_row = _build_constants(c, s, dims, leaf_logits)
    wt = np.ascontiguousarray(W.transpose(1, 0, 2))            # [128, 8, 128]
    mt = np.zeros((128, N_FCHUNKS, 16), dtype=F16)
    mt[:, :, :N_CLASS] = M.transpose(1, 0, 2)

    in_maps = []
    for core in range(N_CORES):
        xc = x[core * B_CORE: (core + 1) * B_CORE]             # [8192, 64]
        xT = np.ascontiguousarray(xc.T).astype(F32)            # [64, 8192]
        x_hi = xT.astype(F16)
        x_lo = (xT - x_hi.astype(F32)).astype(F16)
        xt2 = np.empty((128, B_CORE), dtype=F16)
        xt2[:IN_DIM] = x_hi
        for d, r in lo_row.items():
            xt2[r] = x_lo[d]
        xt2[126] = 1.0
        xt2[127] = 1.0
        in_maps.append({"xt2": xt2, "wt": wt, "mt": mt})

    _CACHE["in_maps"] = in_maps
    nc = _get_program()
    res = run_bass_kernel_spmd(nc, in_maps, core_ids=list(range(N_CORES)))

    out = np.empty((B_FULL, N_CLASS), dtype=F32)
    for core in range(N_CORES):
        outt = res.results[core]["outt"]                       # [128, 2048]
        for sl in range(N_SLABS):
            g, j = divmod(sl, 4)
            blk = outt[32 * j: 32 * j + N_CLASS, g * SLAB: (g + 1) * SLAB]
            out[core * B_CORE + sl * SLAB: core * B_CORE + (sl + 1) * SLAB] = (
                blk.T.astype(F32))
    return out


# revision 20
# speedup vs baseline: 1.0076x; 1.0076x over previous
"""Trainium2 Bass kernel for nn_DFPT_Node (soft binary decision tree).

Full inputs in, full output out; internally data-parallel over 8 NeuronCores
(batch sharded 65536 -> 8 x 8192). Tree params (c, s, dims, leaf_logits) are
baked into compiled constants on the host.

  gate:  g = sigmoid(-4 (x[:,dims] - c)/|s|) = sigmoid(a*x + b) via a scaled
         one-hot matmul with K=128 = [x_hi(64) | x_lo(62 dims) | 1 | 1]; the
         last two rows carry b = b_hi + b_lo (f16 split, ~22 bits), so the
         sigmoid needs no per-chunk bias and one ACT instruction can span
         chunk boundaries (1536-wide supertiles, fewer ACT init charges).
         The two dims whose sharpest gate is softest lose their x_lo row
         (slope <= ~20, error ~1e-3 in z; harmless).
  tree:  levels 0-6 batch-major (batch on partitions), levels 7-9 node-major
         (nodes on partitions, batch on free dim) in block (bit-reversed)
         leaf order; level 9 folded into the output matmul with an 8-chunk
         basis F = [l8, r8, l9a, l9b, u0, u1, q2, q4] (q2 = r9a*g, q4 =
         r9b*g via explicit r9a/r9b subtractions - 2 fewer PSUM chunks than
         the 10-chunk basis at the same DVE op count).
  sched: event-driven software pipeline at chunk granularity: each sigmoid
         supertile completion triggers exactly the newly-unblocked shallow /
         deep / fold work, so the drain after the last sigmoid is short.

Output leaves the device as outT [10->16, B_core] packed 4 slabs per 128
partitions; host transposes back.
"""

import numpy as np

B_FULL = 65536
IN_DIM = 64
N_CLASS = 10
MAX_DEPTH = 10
N_CORES = 8
B_CORE = B_FULL // N_CORES      # 8192
SLAB = 1024                     # batch columns per slab
N_SLABS = B_CORE // SLAB        # 8
N_CHUNKS = 8                    # node-major chunks of 128 nodes
N_FCHUNKS = 8                   # fold basis chunks
SUPER = 1536                    # sigmoid supertile width (3 psum banks)
PAIR_FLAT = 2 * N_CHUNKS * SLAB          # 16384 flat gt elems per slab pair
STEPS_PER_PAIR = (PAIR_FLAT + SUPER - 1) // SUPER  # 11
N_PAIRS = N_SLABS // 2

F16 = np.float16
F32 = np.float32

_CACHE = {}


def _build_tree_layout():
    """pos[d][i] = reference position within level d of block-order index i."""
    pos = [np.array([0], dtype=np.int64)]
    for _ in range(MAX_DEPTH):
        p = pos[-1]
        pos.append(np.concatenate([2 * p, 2 * p + 1]))
    return pos


def _build_constants(c, s, dims, leaf_logits):
    """W chunks [8,128,128] f16 (bias folded in rows 126/127), M [8,128,10]."""
    pos = _build_tree_layout()
    chunk_nodes = -np.ones((N_CHUNKS, 128), dtype=np.int64)
    for d in range(7):
        base = (1 << d) - 1
        chunk_nodes[0, base: base + (1 << d)] = base + pos[d]
    chunk_nodes[1, :] = 127 + pos[7]
    lvl8 = 255 + pos[8]
    chunk_nodes[2, :] = lvl8[:128]
    chunk_nodes[3, :] = lvl8[128:]
    lvl9 = 511 + pos[9]
    for t in range(4):
        chunk_nodes[4 + t, :] = lvl9[128 * t: 128 * (t + 1)]

    a64 = -4.0 / np.abs(s.astype(np.float64))
    a16 = a64.astype(F16)
    b64 = -a16.astype(np.float64) * c.astype(np.float64)
    b_hi = b64.astype(F16)
    b_lo = (b64 - b_hi.astype(np.float64)).astype(F16)

    # the two dims whose sharpest gate is softest lose their x_lo row
    min_s = np.full(IN_DIM, np.inf)
    for g in range(len(dims)):
        d = int(dims[g])
        min_s[d] = min(min_s[d], abs(float(s[g])))
    drop = np.argsort(-min_s)[:2]
    lo_row = {}
    r = IN_DIM
    for d in range(IN_DIM):
        if d not in drop:
            lo_row[d] = r
            r += 1
    assert r == 126

    W = np.zeros((N_CHUNKS, 128, 128), dtype=F16)
    ch_idx, col_idx = np.nonzero(chunk_nodes >= 0)
    g_idx = chunk_nodes[ch_idx, col_idx]
    for ch, col, g in zip(ch_idx, col_idx, g_idx):
        d = int(dims[g])
        W[ch, d, col] = a16[g]
        if d in lo_row:
            W[ch, lo_row[d], col] = a16[g]
        W[ch, 126, col] = b_hi[g]
        W[ch, 127, col] = b_lo[g]

    L_my = leaf_logits[pos[MAX_DEPTH]].astype(np.float64)  # [1024, 10] block
    A = L_my[:512] - L_my[512:]
    Bm = L_my[512:]
    At = [A[128 * t: 128 * (t + 1)] for t in range(4)]
    Bt = [Bm[128 * t: 128 * (t + 1)] for t in range(4)]
    # F basis: [l8, r8, l9a, l9b, u0=l9a*g9a, u1=l9b*g9b, q2=r9a*g9c,
    #           q4=r9b*g9d] with r9a = l8-l9a, r9b = r8-l9b:
    # out = l8 B2 + r8 B3 + l9a (B0-B2) + l9b (B1-B3) + u0 A0 + u1 A1
    #       + q2 A2 + q4 A3
    Mlist = [Bt[2], Bt[3], Bt[0] - Bt[2], Bt[1] - Bt[3],
             At[0], At[1], At[2], At[3]]
    M = np.zeros((N_FCHUNKS, 128, N_CLASS), dtype=F16)
    for i, m in enumerate(Mlist):
        M[i] = m.astype(F16)
    return W, M, lo_row


def _step_table():
    """Per sigmoid step: (gt dest, offset, width, z pieces, completions).

    Phase 1 evaluates chunk 0 (shallow gates) of every slab into gt0 so all
    cascades and transposes run early; phase 2 evaluates chunks 1-7
    slab-major into per-pair gtd tiles. A piece is 512 batch columns
    [h*512, h*512+512) of one chunk of one slab.
    """
    steps = []
    p = 0
    off = 0
    for w in (512, 1536, 1536, 1536, 1536, 1536):
        n = w // 512
        pieces = [(q // 2, 0, q % 2) for q in range(p, p + n)]
        done = [(q // 2, 0) for q in range(p, p + n) if q % 2 == 1]
        steps.append(("gt0", None, off, w, pieces, done))
        p += n
        off += w
    for pair in range(N_PAIRS):
        p = 0
        off = 0
        for k, w in enumerate([1536] * 9 + [512]):
            n = w // 512
            pieces = []
            done = []
            for q in range(p, p + n):
                s_in, r = divmod(q, 14)
                s = 2 * pair + s_in
                ch, h = 1 + r // 2, r % 2
                pieces.append((s, ch, h))
                if h == 1:
                    done.append((s, ch))
            steps.append(("gtd", pair, off, w, pieces, done))
            p += n
            off += w
    return steps


def _build_program():
    import concourse.bass as bass
    import concourse.tile as tile
    from concourse import bacc, mybir

    f16 = mybir.dt.float16
    f32 = mybir.dt.float32
    SIG = mybir.ActivationFunctionType.Sigmoid

    nc = bacc.Bacc("TRN2", target_bir_lowering=False)
    xt2_d = nc.dram_tensor("xt2", [128, B_CORE], f16, kind="ExternalInput")
    wt_d = nc.dram_tensor("wt", [128, N_CHUNKS, 128], f16, kind="ExternalInput")
    mt_d = nc.dram_tensor("mt", [128, N_FCHUNKS, 16], f16, kind="ExternalInput")
    out_d = nc.dram_tensor("outt", [128, B_CORE // 4], f16, kind="ExternalOutput")

    steps = _step_table()

    with tile.TileContext(nc) as tc:
        with (
            tc.tile_pool(name="singles", bufs=1) as singles,
            tc.tile_pool(name="gtpool", bufs=2) as gtpool,
            tc.tile_pool(name="work", bufs=2) as work,
            tc.tile_pool(name="fwork", bufs=2) as fwork,
            tc.tile_pool(name="zpsum", bufs=2, space="PSUM") as zpsum,
            tc.tile_pool(name="opsum", bufs=1, space="PSUM") as opsum,
        ):
            # resident constants; ordered so step 0's z can start ASAP
            w_sb = singles.tile([128, N_CHUNKS, 128], f16)
            nc.sync.dma_start(out=w_sb[:, 0:1, :], in_=wt_d[:, 0:1, :])
            xt2 = singles.tile([128, B_CORE], f16)
            nc.sync.dma_start(out=xt2[:, 0:512], in_=xt2_d[:, 0:512])
            nc.sync.dma_start(out=xt2[:, 512:SLAB], in_=xt2_d[:, 512:SLAB])
            for sl in range(1, N_SLABS):
                t = bass.ts(sl, SLAB)
                nc.sync.dma_start(out=xt2[:, t], in_=xt2_d[:, t])
            nc.sync.dma_start(out=w_sb[:, 1:, :], in_=wt_d[:, 1:, :])
            ones0 = singles.tile([128, 8, 1], f16)
            nc.vector.memset(ones0, 1.0)
            sigwarm = singles.tile([1, 1], f16)
            nc.vector.memset(sigwarm, 0.0)
            nc.scalar.activation(sigwarm, sigwarm, SIG, bias=0.0, scale=1.0)
            m_sb = singles.tile([128, N_FCHUNKS, 16], f16)
            nc.sync.dma_start(out=m_sb, in_=mt_d[:, :, :])
            gt0 = singles.tile([128, N_SLABS * SLAB], f16)

            state = {}

            def emit_shallow(s):
                # chunk-0 gates batch-major via DMA xbar transpose
                gsh = work.tile([128, 8, 128], f16, tag="gsh", name=f"gsh{s}")
                for g in range(8):
                    nc.sync.dma_start(
                        out=gsh[:, g, :],
                        in_=gt0[:, s * SLAB + g * 128: s * SLAB + (g + 1) * 128],
                        transpose=True,
                    )
                # shallow cascade (batch-major, block layout, groups stacked)
                prev = ones0[:, 0:8, :]
                for d in range(7):
                    n = 1 << d
                    cur = work.tile([128, 8, 2 * n], f16, tag=f"pb{d + 1}",
                                    name=f"pb{d + 1}_{s}")
                    gl = gsh[:, :, n - 1: 2 * n - 1]
                    nc.vector.tensor_mul(cur[:, :, 0:n], prev, gl)
                    nc.vector.tensor_sub(cur[:, :, n: 2 * n], prev,
                                         cur[:, :, 0:n])
                    prev = cur
                # p7 batch-major -> node-major via DMA xbar transpose
                p7t = work.tile([128, SLAB], f16, tag="p7t", name=f"p7t{s}")
                for g in range(8):
                    nc.sync.dma_start(
                        out=p7t[:, g * 128: (g + 1) * 128],
                        in_=prev[:, g, :],
                        transpose=True,
                    )
                state[("p7t", s)] = p7t

            def gtc(s, ch):
                gt = state[("gtd", s // 2)]
                base = ((s % 2) * 7 + ch - 1) * SLAB
                return gt[:, base: base + SLAB]

            def ft(name, s):
                t = fwork.tile([128, SLAB], f16, tag=name, name=f"{name}_{s}")
                state[(name, s)] = t
                return t

            FNAMES = ("l8", "r8", "l9a", "l9b", "u0", "u1", "q2", "q4")

            def emit_fold(s, f):
                # fold chunk f of slab s into op[32j:32j+16, h*512:+512]
                grp, j = divmod(s, 4)
                if f == 0 and j == 0:
                    state["op"] = opsum.tile([128, SLAB], f32, tag="op",
                                             name=f"op{grp}")
                op = state["op"]
                src = state[(FNAMES[f], s)]
                for h in range(2):
                    nc.tensor.matmul(
                        op[32 * j: 32 * j + 16, h * 512: h * 512 + 512],
                        lhsT=m_sb[:, f, :],
                        rhs=src[:, h * 512: h * 512 + 512],
                        start=(f == 0),
                        stop=(f == N_FCHUNKS - 1),
                        tile_position=(0, 32 * j),
                    )

            def emit_early_out(grp):
                # slabs 4g..4g+2 (psum rows 0:96) are folded; copy + store
                # them while slab 4g+3 still accumulates rows 96:128
                op = state["op"]
                osb = work.tile([128, SLAB], f16, tag="osb", name=f"osb{grp}")
                state[("osb", grp)] = osb
                nc.vector.tensor_copy(osb[0:96, :], op[0:96, :])
                nc.sync.dma_start(
                    out=out_d[0:96, grp * SLAB: (grp + 1) * SLAB],
                    in_=osb[0:96, :],
                )

            def emit_group_out(grp):
                op = state["op"]
                osb = state[("osb", grp)]
                last = grp == N_SLABS // 4 - 1
                for h in range(2):
                    src = op[96:128, h * 512: h * 512 + 512]
                    dst = osb[96:128, h * 512: h * 512 + 512]
                    if last and h == 1:
                        nc.scalar.copy(dst, src)
                    else:
                        nc.vector.tensor_copy(dst, src)
                    nc.sync.dma_start(
                        out=out_d[96:128, grp * SLAB + h * 512:
                                  grp * SLAB + h * 512 + 512],
                        in_=osb[96:128, h * 512: h * 512 + 512],
                    )

            def handle(s, ch):
                if ch == 0:
                    emit_shallow(s)
                elif ch == 1:
                    p7t = state[("p7t", s)]
                    l8 = ft("l8", s)
                    nc.vector.tensor_mul(l8, p7t, gtc(s, 1))
                    r8 = ft("r8", s)
                    nc.vector.tensor_sub(r8, p7t, l8)
                elif ch == 2:
                    l8 = state[("l8", s)]
                    l9a = ft("l9a", s)
                    nc.vector.tensor_mul(l9a, l8, gtc(s, 2))
                    r9a = ft("r9a", s)
                    nc.vector.tensor_sub(r9a, l8, l9a)
                    emit_fold(s, 0)
                    emit_fold(s, 1)
                elif ch == 3:
                    r8 = state[("r8", s)]
                    l9b = ft("l9b", s)
                    nc.vector.tensor_mul(l9b, r8, gtc(s, 3))
                    r9b = ft("r9b", s)
                    nc.vector.tensor_sub(r9b, r8, l9b)
                    emit_fold(s, 2)
                elif ch == 4:
                    u0 = ft("u0", s)
                    nc.gpsimd.tensor_mul(u0, state[("l9a", s)], gtc(s, 4))
                    emit_fold(s, 3)
                elif ch == 5:
                    u1 = ft("u1", s)
                    nc.gpsimd.tensor_mul(u1, state[("l9b", s)], gtc(s, 5))
                    emit_fold(s, 4)
                elif ch == 6:
                    q2 = ft("q2", s)
                    nc.vector.tensor_mul(q2, state[("r9a", s)], gtc(s, 6))
                    emit_fold(s, 5)
                elif ch == 7:
                    q4 = ft("q4", s)
                    nc.vector.tensor_mul(q4, state[("r9b", s)], gtc(s, 7))
                    emit_fold(s, 6)
                    state["fold7"] = s

            deferred = {}

            def flush_fold7(ti):
                s = state.pop("fold7", None)
                if s is not None:
                    emit_fold(s, 7)
                    if s % 4 == 2:
                        # defer the early copy two steps so it sits behind
                        # the next slab's deep ops in the DVE queue
                        deferred.setdefault(ti + 2, []).append(
                            lambda g=s // 4: emit_early_out(g))
                    elif s % 4 == 3:
                        emit_group_out(s // 4)

            for ti, (dest, pair, off, width, pieces, done) in enumerate(steps):
                with tc.high_priority():
                    if dest == "gt0":
                        gt = gt0
                    else:
                        if off == 0:
                            state[("gtd", pair)] = gtpool.tile(
                                [128, 2 * 7 * SLAB], f16, tag="gtd",
                                name=f"gtd{pair}")
                        gt = state[("gtd", pair)]
                    zs = zpsum.tile([128, SUPER], f32, tag="zs",
                                    name=f"zs{ti}")
                    for i, (s, ch, h) in enumerate(pieces):
                        col0 = s * SLAB + h * 512
                        nc.tensor.matmul(
                            zs[:, i * 512: (i + 1) * 512],
                            lhsT=w_sb[:, ch, :],
                            rhs=xt2[:, col0: col0 + 512],
                            start=True,
                            stop=True,
                        )
                    nc.scalar.activation(
                        gt[:, off: off + width],
                        zs[:, 0:width], SIG, bias=0.0, scale=1.0,
                    )
                flush_fold7(ti)
                for s, ch in done:
                    handle(s, ch)
                for fn in deferred.pop(ti, []):
                    fn()
            for fns in deferred.values():
                for fn in fns:
                    fn()
            flush_fold7(len(steps))

    nc.finalize()
    return nc


def _get_program():
    if "nc" not in _CACHE:
        _CACHE["nc"] = _build_program()
    return _CACHE["nc"]


def kernel(x, c, s, leaf_logits, dims, max_depth):
    from concourse.bass_utils import run_bass_kernel_spmd

    assert int(max_depth) == MAX_DEPTH
    x = np.asarray(x, dtype=F32)
    c = np.asarray(c, dtype=F32)
    s = np.asarray(s, dtype=F32)
    leaf_logits = np.asarray(leaf_logits, dtype=F32)
    dims = np.asarray(dims)

    W, M, lo_row = _build_constants(c, s, dims, leaf_logits)
    wt = np.ascontiguousarray(W.transpose(1, 0, 2))            # [128, 8, 128]
    mt = np.zeros((128, N_FCHUNKS, 16), dtype=F16)
    mt[:, :, :N_CLASS] = M.transpose(1, 0, 2)

    in_maps = []
    for core in range(N_CORES):
        xc = x[core * B_CORE: (core + 1) * B_CORE]             # [8192, 64]
        xT = np.ascontiguousarray(xc.T).astype(F32)            # [64, 8192]
        x_hi = xT.astype(F16)
        x_lo = (xT - x_hi.astype(F32)).astype(F16)
        xt2 = np.empty((128, B_CORE), dtype=F16)
        xt2[:IN_DIM] = x_hi
        for d, r in lo_row.items():
            xt2[r] = x_lo[d]
        xt2[126] = 1.0
        xt2[127] = 1.0
        in_maps.append({"xt2": xt2, "wt": wt, "mt": mt})

    _CACHE["in_maps"] = in_maps
    nc = _get_program()
    res = run_bass_kernel_spmd(nc, in_maps, core_ids=list(range(N_CORES)))

    out = np.empty((B_FULL, N_CLASS), dtype=F32)
    for core in range(N_CORES):
        outt = res.results[core]["outt"]                       # [128, 2048]
        for sl in range(N_SLABS):
            g, j = divmod(sl, 4)
            blk = outt[32 * j: 32 * j + N_CLASS, g * SLAB: (g + 1) * SLAB]
            out[core * B_CORE + sl * SLAB: core * B_CORE + (sl + 1) * SLAB] = (
                blk.T.astype(F32))
    return out


# revision 21
# speedup vs baseline: 1.0436x; 1.0358x over previous
"""Trainium2 Bass kernel for nn_DFPT_Node (soft binary decision tree).

Full inputs in, full output out; internally data-parallel over 8 NeuronCores
(batch sharded 65536 -> 8 x 8192). Tree params (c, s, dims, leaf_logits) are
baked into compiled constants on the host.

  gate:  g = sigmoid(-4 (x[:,dims] - c)/|s|) = sigmoid(a*x + b) via a scaled
         one-hot matmul with K=128 = [x_hi(64) | x_lo(62 dims) | 1 | 1]; the
         last two rows carry b = b_hi + b_lo (f16 split, ~22 bits), so the
         sigmoid needs no per-chunk bias and one ACT instruction can span
         chunk boundaries (1536-wide supertiles, fewer ACT init charges).
         The two dims whose sharpest gate is softest lose their x_lo row
         (slope <= ~20, error ~1e-3 in z; harmless).
  tree:  levels 0-6 batch-major (batch on partitions), levels 7-9 node-major
         (nodes on partitions, batch on free dim) in block (bit-reversed)
         leaf order; level 9 folded into the output matmul with an 8-chunk
         basis F = [l8, r8, l9a, l9b, u0, u1, q2, q4] (q2 = r9a*g, q4 =
         r9b*g via explicit r9a/r9b subtractions - 2 fewer PSUM chunks than
         the 10-chunk basis at the same DVE op count).
  sched: event-driven software pipeline at chunk granularity: each sigmoid
         supertile completion triggers exactly the newly-unblocked shallow /
         deep / fold work, so the drain after the last sigmoid is short.

Output leaves the device as outT [10->16, B_core] packed 4 slabs per 128
partitions; host transposes back.
"""

import numpy as np

B_FULL = 65536
IN_DIM = 64
N_CLASS = 10
MAX_DEPTH = 10
N_CORES = 8
B_CORE = B_FULL // N_CORES      # 8192
SLAB = 1024                     # batch columns per slab
N_SLABS = B_CORE // SLAB        # 8
N_CHUNKS = 8                    # node-major chunks of 128 nodes
N_FCHUNKS = 8                   # fold basis chunks
SUPER = 1536                    # sigmoid supertile width (3 psum banks)
PAIR_FLAT = 2 * N_CHUNKS * SLAB          # 16384 flat gt elems per slab pair
STEPS_PER_PAIR = (PAIR_FLAT + SUPER - 1) // SUPER  # 11
N_PAIRS = N_SLABS // 2

F16 = np.float16
F32 = np.float32

_CACHE = {}


def _build_tree_layout():
    """pos[d][i] = reference position within level d of block-order index i."""
    pos = [np.array([0], dtype=np.int64)]
    for _ in range(MAX_DEPTH):
        p = pos[-1]
        pos.append(np.concatenate([2 * p, 2 * p + 1]))
    return pos


def _build_constants(c, s, dims, leaf_logits):
    """W chunks [8,128,128] f16 (bias folded in rows 126/127), M [8,128,10]."""
    pos = _build_tree_layout()
    chunk_nodes = -np.ones((N_CHUNKS, 128), dtype=np.int64)
    for d in range(7):
        base = (1 << d) - 1
        chunk_nodes[0, base: base + (1 << d)] = base + pos[d]
    chunk_nodes[1, :] = 127 + pos[7]
    lvl8 = 255 + pos[8]
    chunk_nodes[2, :] = lvl8[:128]
    chunk_nodes[3, :] = lvl8[128:]
    lvl9 = 511 + pos[9]
    for t in range(4):
        chunk_nodes[4 + t, :] = lvl9[128 * t: 128 * (t + 1)]

    a64 = -4.0 / np.abs(s.astype(np.float64))
    a16 = a64.astype(F16)
    b64 = -a16.astype(np.float64) * c.astype(np.float64)
    b_hi = b64.astype(F16)
    b_lo = (b64 - b_hi.astype(np.float64)).astype(F16)

    # the two dims whose sharpest gate is softest lose their x_lo row
    min_s = np.full(IN_DIM, np.inf)
    for g in range(len(dims)):
        d = int(dims[g])
        min_s[d] = min(min_s[d], abs(float(s[g])))
    drop = np.argsort(-min_s)[:2]
    lo_row = {}
    r = IN_DIM
    for d in range(IN_DIM):
        if d not in drop:
            lo_row[d] = r
            r += 1
    assert r == 126

    W = np.zeros((N_CHUNKS, 128, 128), dtype=F16)
    ch_idx, col_idx = np.nonzero(chunk_nodes >= 0)
    g_idx = chunk_nodes[ch_idx, col_idx]
    for ch, col, g in zip(ch_idx, col_idx, g_idx):
        d = int(dims[g])
        W[ch, d, col] = a16[g]
        if d in lo_row:
            W[ch, lo_row[d], col] = a16[g]
        W[ch, 126, col] = b_hi[g]
        W[ch, 127, col] = b_lo[g]

    L_my = leaf_logits[pos[MAX_DEPTH]].astype(np.float64)  # [1024, 10] block
    A = L_my[:512] - L_my[512:]
    Bm = L_my[512:]
    At = [A[128 * t: 128 * (t + 1)] for t in range(4)]
    Bt = [Bm[128 * t: 128 * (t + 1)] for t in range(4)]
    # F basis: [l8, r8, l9a, l9b, u0=l9a*g9a, u1=l9b*g9b, q2=r9a*g9c,
    #           q4=r9b*g9d] with r9a = l8-l9a, r9b = r8-l9b:
    # out = l8 B2 + r8 B3 + l9a (B0-B2) + l9b (B1-B3) + u0 A0 + u1 A1
    #       + q2 A2 + q4 A3
    Mlist = [Bt[2], Bt[3], Bt[0] - Bt[2], Bt[1] - Bt[3],
             At[0], At[1], At[2], At[3]]
    M = np.zeros((N_FCHUNKS, 128, N_CLASS), dtype=F16)
    for i, m in enumerate(Mlist):
        M[i] = m.astype(F16)
    return W, M, lo_row


def _step_table():
    """Per sigmoid step: (gt dest, offset, width, z pieces, completions).

    Phase 1 evaluates chunk 0 (shallow gates) of every slab into gt0 so all
    cascades and transposes run early; phase 2 evaluates chunks 1-7
    slab-major into per-pair gtd tiles. A piece is 512 batch columns
    [h*512, h*512+512) of one chunk of one slab.
    """
    steps = []
    p = 0
    off = 0
    for w in (512, 1536, 1536, 1536, 1536, 1536):
        n = w // 512
        pieces = [(q // 2, 0, q % 2) for q in range(p, p + n)]
        done = [(q // 2, 0) for q in range(p, p + n) if q % 2 == 1]
        steps.append(("gt0", None, off, w, pieces, done))
        p += n
        off += w
    for pair in range(N_PAIRS):
        p = 0
        off = 0
        for k, w in enumerate([1536] * 9 + [512]):
            n = w // 512
            pieces = []
            done = []
            for q in range(p, p + n):
                s_in, r = divmod(q, 14)
                s = 2 * pair + s_in
                ch, h = 1 + r // 2, r % 2
                pieces.append((s, ch, h))
                if h == 1:
                    done.append((s, ch))
            steps.append(("gtd", pair, off, w, pieces, done))
            p += n
            off += w
    return steps


def _build_program():
    import concourse.bass as bass
    import concourse.tile as tile
    from concourse import bacc, mybir

    f16 = mybir.dt.float16
    f32 = mybir.dt.float32
    SIG = mybir.ActivationFunctionType.Sigmoid

    nc = bacc.Bacc("TRN2", target_bir_lowering=False)
    xt2_d = nc.dram_tensor("xt2", [128, B_CORE], f16, kind="ExternalInput")
    wt_d = nc.dram_tensor("wt", [128, N_CHUNKS, 128], f16, kind="ExternalInput")
    mt_d = nc.dram_tensor("mt", [128, N_FCHUNKS, 16], f16, kind="ExternalInput")
    out_d = nc.dram_tensor("outt", [128, B_CORE // 4], f16, kind="ExternalOutput")

    steps = _step_table()

    with tile.TileContext(nc) as tc:
        with (
            tc.tile_pool(name="singles", bufs=1) as singles,
            tc.tile_pool(name="gtpool", bufs=3) as gtpool,
            tc.tile_pool(name="work", bufs=2) as work,
            tc.tile_pool(name="fwork", bufs=2) as fwork,
            tc.tile_pool(name="zpsum", bufs=2, space="PSUM") as zpsum,
            tc.tile_pool(name="opsum", bufs=1, space="PSUM") as opsum,
        ):
            # resident constants; ordered so step 0's z can start ASAP
            w_sb = singles.tile([128, N_CHUNKS, 128], f16)
            nc.sync.dma_start(out=w_sb[:, 0:1, :], in_=wt_d[:, 0:1, :])
            xt2 = singles.tile([128, B_CORE], f16)
            nc.sync.dma_start(out=xt2[:, 0:512], in_=xt2_d[:, 0:512])
            nc.sync.dma_start(out=xt2[:, 512:SLAB], in_=xt2_d[:, 512:SLAB])
            for sl in range(1, N_SLABS):
                t = bass.ts(sl, SLAB)
                nc.sync.dma_start(out=xt2[:, t], in_=xt2_d[:, t])
            nc.sync.dma_start(out=w_sb[:, 1:, :], in_=wt_d[:, 1:, :])
            ones0 = singles.tile([128, 8, 1], f16)
            nc.vector.memset(ones0, 1.0)
            sigwarm = singles.tile([1, 1], f16)
            nc.vector.memset(sigwarm, 0.0)
            nc.scalar.activation(sigwarm, sigwarm, SIG, bias=0.0, scale=1.0)
            m_sb = singles.tile([128, N_FCHUNKS, 16], f16)
            nc.sync.dma_start(out=m_sb, in_=mt_d[:, :, :])
            gt0 = singles.tile([128, N_SLABS * SLAB], f16)

            state = {}

            def emit_shallow(s):
                # chunk-0 gates batch-major via DMA xbar transpose
                gsh = work.tile([128, 8, 128], f16, tag="gsh", name=f"gsh{s}")
                for g in range(8):
                    nc.sync.dma_start(
                        out=gsh[:, g, :],
                        in_=gt0[:, s * SLAB + g * 128: s * SLAB + (g + 1) * 128],
                        transpose=True,
                    )
                # shallow cascade (batch-major, block layout, groups stacked)
                prev = ones0[:, 0:8, :]
                for d in range(7):
                    n = 1 << d
                    cur = work.tile([128, 8, 2 * n], f16, tag=f"pb{d + 1}",
                                    name=f"pb{d + 1}_{s}")
                    gl = gsh[:, :, n - 1: 2 * n - 1]
                    nc.vector.tensor_mul(cur[:, :, 0:n], prev, gl)
                    nc.vector.tensor_sub(cur[:, :, n: 2 * n], prev,
                                         cur[:, :, 0:n])
                    prev = cur
                # p7 batch-major -> node-major via DMA xbar transpose
                p7t = work.tile([128, SLAB], f16, tag="p7t", name=f"p7t{s}")
                for g in range(8):
                    nc.sync.dma_start(
                        out=p7t[:, g * 128: (g + 1) * 128],
                        in_=prev[:, g, :],
                        transpose=True,
                    )
                state[("p7t", s)] = p7t

            def gtc(s, ch):
                gt = state[("gtd", s // 2)]
                base = ((s % 2) * 7 + ch - 1) * SLAB
                return gt[:, base: base + SLAB]

            def ft(name, s):
                t = fwork.tile([128, SLAB], f16, tag=name, name=f"{name}_{s}")
                state[(name, s)] = t
                return t

            FNAMES = ("l8", "r8", "l9a", "l9b", "u0", "u1", "q2", "q4")

            def emit_fold(s, f):
                # fold chunk f of slab s into op_h[32j:32j+16, :]; the two
                # column halves live in separate psum tiles so the final
                # copies of each half start right after that half's last fold
                grp, j = divmod(s, 4)
                if f == 0 and j == 0:
                    for h in range(2):
                        state["op", h] = opsum.tile(
                            [128, 512], f32, tag=f"op{h}", name=f"op{grp}_{h}")
                src = state[(FNAMES[f], s)]
                for h in range(2):
                    nc.tensor.matmul(
                        state["op", h][32 * j: 32 * j + 16, :],
                        lhsT=m_sb[:, f, :],
                        rhs=src[:, h * 512: h * 512 + 512],
                        start=(f == 0),
                        stop=(f == N_FCHUNKS - 1),
                        tile_position=(0, 32 * j),
                    )

            def emit_group_out(grp):
                last = grp == N_SLABS // 4 - 1
                for h in range(2):
                    osb = work.tile([128, 512], f16, tag="osb",
                                    name=f"osb{grp}_{h}")
                    if last and h == 1:
                        nc.scalar.copy(osb, state["op", h])
                    else:
                        nc.vector.tensor_copy(osb, state["op", h])
                    nc.sync.dma_start(
                        out=out_d[:, grp * SLAB + h * 512:
                                  grp * SLAB + h * 512 + 512],
                        in_=osb,
                    )

            def handle(s, ch):
                if ch == 0:
                    emit_shallow(s)
                elif ch == 1:
                    p7t = state[("p7t", s)]
                    l8 = ft("l8", s)
                    nc.vector.tensor_mul(l8, p7t, gtc(s, 1))
                    r8 = ft("r8", s)
                    nc.vector.tensor_sub(r8, p7t, l8)
                elif ch == 2:
                    l8 = state[("l8", s)]
                    l9a = ft("l9a", s)
                    nc.vector.tensor_mul(l9a, l8, gtc(s, 2))
                    r9a = ft("r9a", s)
                    nc.vector.tensor_sub(r9a, l8, l9a)
                    emit_fold(s, 0)
                    emit_fold(s, 1)
                elif ch == 3:
                    r8 = state[("r8", s)]
                    l9b = ft("l9b", s)
                    nc.vector.tensor_mul(l9b, r8, gtc(s, 3))
                    r9b = ft("r9b", s)
                    nc.vector.tensor_sub(r9b, r8, l9b)
                    emit_fold(s, 2)
                elif ch == 4:
                    u0 = ft("u0", s)
                    nc.gpsimd.tensor_mul(u0, state[("l9a", s)], gtc(s, 4))
                    emit_fold(s, 3)
                elif ch == 5:
                    u1 = ft("u1", s)
                    nc.gpsimd.tensor_mul(u1, state[("l9b", s)], gtc(s, 5))
                    emit_fold(s, 4)
                elif ch == 6:
                    q2 = ft("q2", s)
                    nc.vector.tensor_mul(q2, state[("r9a", s)], gtc(s, 6))
                    emit_fold(s, 5)
                elif ch == 7:
                    q4 = ft("q4", s)
                    nc.vector.tensor_mul(q4, state[("r9b", s)], gtc(s, 7))
                    emit_fold(s, 6)
                    state["fold7"] = s

            def flush_fold7():
                s = state.pop("fold7", None)
                if s is not None:
                    emit_fold(s, 7)
                    if s % 4 == 3:
                        emit_group_out(s // 4)

            for ti, (dest, pair, off, width, pieces, done) in enumerate(steps):
                with tc.high_priority():
                    if dest == "gt0":
                        gt = gt0
                    else:
                        if off == 0:
                            state[("gtd", pair)] = gtpool.tile(
                                [128, 2 * 7 * SLAB], f16, tag="gtd",
                                name=f"gtd{pair}")
                        gt = state[("gtd", pair)]
                    zs = zpsum.tile([128, SUPER], f32, tag="zs",
                                    name=f"zs{ti}")
                    for i, (s, ch, h) in enumerate(pieces):
                        col0 = s * SLAB + h * 512
                        nc.tensor.matmul(
                            zs[:, i * 512: (i + 1) * 512],
                            lhsT=w_sb[:, ch, :],
                            rhs=xt2[:, col0: col0 + 512],
                            start=True,
                            stop=True,
                        )
                    nc.scalar.activation(
                        gt[:, off: off + width],
                        zs[:, 0:width], SIG, bias=0.0, scale=1.0,
                    )
                flush_fold7()
                for s, ch in done:
                    handle(s, ch)
            flush_fold7()

    nc.finalize()
    return nc


def _get_program():
    if "nc" not in _CACHE:
        _CACHE["nc"] = _build_program()
    return _CACHE["nc"]


def kernel(x, c, s, leaf_logits, dims, max_depth):
    from concourse.bass_utils import run_bass_kernel_spmd

    assert int(max_depth) == MAX_DEPTH
    x = np.asarray(x, dtype=F32)
    c = np.asarray(c, dtype=F32)
    s = np.asarray(s, dtype=F32)
    leaf_logits = np.asarray(leaf_logits, dtype=F32)
    dims = np.asarray(dims)

    W, M, lo_row = _build_constants(c, s, dims, leaf_logits)
    wt = np.ascontiguousarray(W.transpose(1, 0, 2))            # [128, 8, 128]
    mt = np.zeros((128, N_FCHUNKS, 16), dtype=F16)
    mt[:, :, :N_CLASS] = M.transpose(1, 0, 2)

    in_maps = []
    for core in range(N_CORES):
        xc = x[core * B_CORE: (core + 1) * B_CORE]             # [8192, 64]
        xT = np.ascontiguousarray(xc.T).astype(F32)            # [64, 8192]
        x_hi = xT.astype(F16)
        x_lo = (xT - x_hi.astype(F32)).astype(F16)
        xt2 = np.empty((128, B_CORE), dtype=F16)
        xt2[:IN_DIM] = x_hi
        for d, r in lo_row.items():
            xt2[r] = x_lo[d]
        xt2[126] = 1.0
        xt2[127] = 1.0
        in_maps.append({"xt2": xt2, "wt": wt, "mt": mt})

    _CACHE["in_maps"] = in_maps
    nc = _get_program()
    res = run_bass_kernel_spmd(nc, in_maps, core_ids=list(range(N_CORES)))

    out = np.empty((B_FULL, N_CLASS), dtype=F32)
    for core in range(N_CORES):
        outt = res.results[core]["outt"]                       # [128, 2048]
        for sl in range(N_SLABS):
            g, j = divmod(sl, 4)
            blk = outt[32 * j: 32 * j + N_CLASS, g * SLAB: (g + 1) * SLAB]
            out[core * B_CORE + sl * SLAB: core * B_CORE + (sl + 1) * SLAB] = (
                blk.T.astype(F32))
    return out
